# revision 3
# baseline (speedup 1.0000x reference)
"""Trainium2 Bass kernel for nn_BDH_90984587198975 (6-layer BDH with Hebbian
fast weights), SPMD over 8 NeuronCores.

Sharding: tensor-parallel over the flattened latent dim NHL=4*8192.  Core c
owns a 4096-wide slice of head h=c//2 (half=c%2), with lanes permuted so rope
pairs split into [even-members(2048) | odd-members(2048)] (rotation becomes a
tile swap instead of a cross-partition shuffle).  F (fast weights) stays
sharded by latent rows — its update is local.  Per layer there are exactly two
collectives: a pair AllReduce of the attention partial (L split in half within
a head) and an 8-core AllReduce of the y_mlp partial (decoder/F/Hebbian terms
contract over the latent shard).

All matmuls run in bf16 (f32 accumulation in PSUM); LayerNorm statistics and
the residual stream stay f32.
"""
import math
import numpy as np
import ml_dtypes

import concourse.bass as bass
import concourse.mybir as mybir
import concourse.tile as tile
from concourse import bacc
from concourse.masks import make_identity
from concourse.bass_utils import run_bass_kernel_spmd

BF = ml_dtypes.bfloat16
f32 = mybir.dt.float32
bf16 = mybir.dt.bfloat16
AF = mybir.ActivationFunctionType
OP = mybir.AluOpType

N_LAYER = 6
D = 256
NH = 4
VOCAB = 130
LR = 0.01
L = 8192
EPS = 1e-5
TWO_PI = 2.0 * math.pi
THETA = 65536.0
B, T = 2, 512
NCORE = 8
SH = 4096          # latent shard per core
NLT = SH // 128    # 32 latent tiles
NBT = (B * T) // 128  # 8 bt tiles
CHK = 128          # hebbian time chunk
NCH = T // CHK     # 4 chunks

_CACHE = {}


# ----------------------------------------------------------------- builder --
def _emit(nc, n_layer, taps, ablate=None):
    # ---- DRAM I/O ----
    oh = nc.dram_tensor("onehotT", [256, 1024], f32, kind="ExternalInput")
    emb = nc.dram_tensor("embedp", [256, 256], f32, kind="ExternalInput")
    wenc = nc.dram_tensor("wenc", [NLT * 256, 128], bf16, kind="ExternalInput")
    wencv = nc.dram_tensor("wencv", [NLT * 256, 128], bf16, kind="ExternalInput")
    wdec = nc.dram_tensor("wdec", [SH, 256], bf16, kind="ExternalInput")
    trig = nc.dram_tensor("trig", [2048, 1024], bf16, kind="ExternalInput")
    msk = nc.dram_tensor("masks", [128, 384], f32, kind="ExternalInput")
    lmh = nc.dram_tensor("lmh", [256, 130], bf16, kind="ExternalInput")
    out = nc.dram_tensor("out", [1024, 130], f32, kind="ExternalOutput")
    tap_t = {}
    if taps:
        for name, shape, dt_ in [("t_x0", [128, 2048], f32), ("t_xs", [128, 1024], bf16),
                                 ("t_attn", [128, 2048], bf16), ("t_ymlp", [128, 2048], bf16),
                                 ("t_x1", [128, 2048], f32), ("t_f", [128, 8192], bf16)]:
            tap_t[name] = nc.dram_tensor(name, shape, dt_, kind="ExternalOutput")

    from contextlib import ExitStack
    tc = tile.TileContext(nc)
    with tc, ExitStack() as stk:
        per = stk.enter_context(tc.tile_pool(name="per", bufs=1))
        sm = stk.enter_context(tc.tile_pool(name="sm", bufs=2))
        qp = stk.enter_context(tc.tile_pool(name="qp", bufs=4))
        ysp = stk.enter_context(tc.tile_pool(name="ysp", bufs=3))
        wp = stk.enter_context(tc.tile_pool(name="wp", bufs=4))
        tp = stk.enter_context(tc.tile_pool(name="tp", bufs=3))
        ps = stk.enter_context(tc.tile_pool(name="ps", bufs=4, space="PSUM"))
        pacc = stk.enter_context(tc.tile_pool(name="pacc", bufs=4, space="PSUM"))
        dram = stk.enter_context(tc.tile_pool(name="dram", bufs=2, space="DRAM"))

        # ---- persistent tiles ----
        xs = per.tile([128, NLT * 1024], bf16, tag="xs")
        F = per.tile([128, NLT * 256], bf16, tag="F")
        xf = per.tile([128, 2048], f32, tag="xf")
        xbf = per.tile([128, 2048], bf16, tag="xbf")
        xTbf = per.tile([128, 2048], bf16, tag="xTbf")
        ymlp = per.tile([128, 2048], bf16, tag="ymlp")
        attn = per.tile([128, 2048], bf16, tag="attn")
        ykv = per.tile([128, 2048], bf16, tag="ykv")
        ykvT = per.tile([128, 2048], bf16, tag="ykvT")
        ident = per.tile([128, 128], bf16, tag="ident")
        identf = per.tile([128, 128], f32, tag="identf")
        maskU = per.tile([128, 128], f32, tag="maskU")
        maskS = per.tile([128, 128], f32, tag="maskS")
        maskS0 = per.tile([128, 128], f32, tag="maskS0")
        epst = per.tile([128, 1], f32, tag="epst")
        xsh = [per.tile([128, 256], bf16, tag=f"xsh{c}", name=f"xsh{c}") for c in range(B)]
        xbt = [per.tile([128, SH], bf16, tag=f"xbt{c}", name=f"xbt{c}") for c in range(B)]
        scsb = [per.tile([128, 1280], bf16, tag=f"scsb{b}", name=f"scsb{b}") for b in range(B)]
        ssb = [per.tile([128, 256], bf16, tag=f"ssb{b}", name=f"ssb{b}") for b in range(B)]

        make_identity(nc, ident[:])
        make_identity(nc, identf[:])
        nc.vector.memset(epst[:], EPS)
        nc.sync.dma_start(maskU[:], msk[:, 0:128])
        nc.sync.dma_start(maskS[:], msk[:, 128:256])
        nc.sync.dma_start(maskS0[:], msk[:, 256:384])
        nc.vector.memset(F[:], 0.0)

        # ---------------- helpers ----------------
        def ln_stats(src_ap, pref):
            """mean/rstd along free dim (256) of [128,256] src; returns
            (nmean, rstd) [128,1] f32 tiles."""
            ssum = sm.tile([128, 1], f32, tag="ssum", bufs=2)
            nmean = sm.tile([128, 1], f32, tag="nmean", bufs=2)
            cent = sm.tile([128, 256], f32, tag="cent", bufs=2)
            sq = sm.tile([128, 256], f32, tag="sq", bufs=2)
            sqsum = sm.tile([128, 1], f32, tag="sqsum", bufs=2)
            std = sm.tile([128, 1], f32, tag="std", bufs=2)
            rstd = sm.tile([128, 1], f32, tag="rstd", bufs=2)
            nc.vector.tensor_reduce(ssum[:], src_ap, axis=mybir.AxisListType.X,
                                    op=OP.add)
            nc.vector.tensor_scalar_mul(nmean[:], ssum[:], -1.0 / 256.0)
            nc.scalar.activation(cent[:], src_ap, AF.Identity, bias=nmean[:])
            nc.scalar.activation(sq[:], cent[:], AF.Square, accum_out=sqsum[:])
            nc.scalar.activation(std[:], sqsum[:], AF.Sqrt, scale=1.0 / 256.0,
                                 bias=epst[:])
            nc.vector.reciprocal(rstd[:], std[:])
            return cent, rstd

        def emit_ln(src_ap, outs):
            """LN over free dim; writes each (ap, ) in outs (dtype via ap)."""
            cent, rstd = ln_stats(src_ap, "")
            for oap in outs:
                nc.scalar.activation(oap, cent[:], AF.Copy, scale=rstd[:])

        def transpose_128(dst_ap, src_ap, dt):
            """PE transpose src [128,128] -> psum -> dst (dtype dt)."""
            pt = ps.tile([128, 512], dt, tag="hot1", bufs=2, name="pt")
            idn = ident if dt == bf16 else identf
            nc.tensor.transpose(pt[:, 0:128], src_ap, idn[:])
            nc.vector.tensor_copy(dst_ap, pt[:, 0:128])

        def make_xT():
            """xTbf[dh*1024 + bt] = x^T from xf."""
            for j in range(NBT):
                for dh in range(2):
                    pt = ps.tile([128, 512], f32, tag="hot1", bufs=2, name="pt")
                    nc.tensor.transpose(
                        pt[:, 0:128], xf[:, j * 256 + dh * 128:j * 256 + dh * 128 + 128],
                        identf[:])
                    nc.vector.tensor_copy(
                        xTbf[:, dh * 1024 + j * 128:dh * 1024 + j * 128 + 128],
                        pt[:, 0:128])

        # ---------------- embedding ----------------
        ohsb = [sm.tile([128, 1024], f32, tag=f"oh{v}", bufs=1, name=f"oh{v}") for v in range(2)]
        embsb = [sm.tile([128, 256], f32, tag=f"em{v}", bufs=1, name=f"em{v}") for v in range(2)]
        for v in range(2):
            nc.sync.dma_start(ohsb[v][:], oh[v * 128:(v + 1) * 128, :])
            nc.sync.dma_start(embsb[v][:], emb[v * 128:(v + 1) * 128, :])
        for j in range(NBT):
            pe = ps.tile([128, 512], f32, tag="hot0", bufs=2, name="pe")
            for v in range(2):
                nc.tensor.matmul(pe[:, 0:256], ohsb[v][:, j * 128:(j + 1) * 128],
                                 embsb[v][:], start=(v == 0), stop=(v == 1))
            emit_ln(pe[:, 0:256],
                    [xf[:, j * 256:(j + 1) * 256], xbf[:, j * 256:(j + 1) * 256]])
        make_xT()
        if taps:
            nc.sync.dma_start(tap_t["t_x0"][:], xf[:])

        # ---------------- layers ----------------
        for lay in range(n_layer):
            last = lay == n_layer - 1

            # -- P1: xs^T = relu(wenc^T x) --
            for lt in range(NLT):
                wt = wp.tile([128, 256], bf16, tag="wenc")
                for dh in range(2):
                    nc.sync.dma_start(wt[:, dh * 128:(dh + 1) * 128],
                                      wenc[(lt * 2 + dh) * 128:(lt * 2 + dh + 1) * 128, :])
                for bh in range(2):  # bt halves (512 cols each)
                    pp = ps.tile([128, 512], f32, tag="hot0", bufs=2, name="pp")
                    for dh in range(2):
                        nc.tensor.matmul(
                            pp[:], wt[:, dh * 128:(dh + 1) * 128],
                            xTbf[:, dh * 1024 + bh * 512:dh * 1024 + bh * 512 + 512],
                            start=(dh == 0), stop=(dh == 1))
                    dst = xs[:, lt * 1024 + bh * 512:lt * 1024 + bh * 512 + 512]
                    if lt % 2 == 0:
                        nc.scalar.activation(dst, pp[:], AF.Relu)
                    else:
                        nc.vector.tensor_scalar_max(dst, pp[:], 0.0)
            if taps and lay == 0:
                nc.sync.dma_start(tap_t["t_xs"][:], xs[:, 0:1024])

            # -- P2+P3: rope -> scores -> attn halves (per b) --
            for b in ([] if ablate == "att" else range(B)):
                psc = [pacc.tile([128, 512], f32, tag=f"acc{u}", bufs=1, name=f"psc{u}") for u in range(4)]
                for pt in range(16):  # pair tiles
                    tg = tp.tile([128, 1024], bf16, tag="trig")
                    nc.sync.dma_start(tg[:], trig[pt * 128:(pt + 1) * 128, :])
                    ct, st = tg[:, 0:512], tg[:, 512:1024]
                    xe = xs[:, pt * 1024 + b * 512:pt * 1024 + b * 512 + 512]
                    xo = xs[:, (16 + pt) * 1024 + b * 512:(16 + pt) * 1024 + b * 512 + 512]
                    t1 = sm.tile([128, 512], bf16, tag="ropet1", bufs=2)
                    t2 = sm.tile([128, 512], bf16, tag="ropet2", bufs=2)
                    t3 = sm.tile([128, 512], bf16, tag="ropet3", bufs=2)
                    t4 = sm.tile([128, 512], bf16, tag="ropet4", bufs=2)
                    qe = qp.tile([128, 512], bf16, tag="q")
                    qo = qp.tile([128, 512], bf16, tag="q")
                    if ablate == "rope":
                        nc.vector.tensor_copy(qe[:], xe)
                        nc.vector.tensor_copy(qo[:], xo)
                    else:
                        nc.vector.tensor_tensor(t1[:], xe, ct, op=OP.mult)
                        nc.vector.tensor_tensor(t2[:], xo, st, op=OP.mult)
                        nc.vector.tensor_tensor(qe[:], t1[:], t2[:], op=OP.subtract)
                        nc.gpsimd.tensor_tensor(t3[:], xo, ct, op=OP.mult)
                        nc.gpsimd.tensor_tensor(t4[:], xe, st, op=OP.mult)
                        nc.gpsimd.tensor_tensor(qo[:], t3[:], t4[:], op=OP.add)
                    for qt, lt in ((qe, pt), (qo, 16 + pt)):
                        for ut in range(4):
                            n = 512 - ut * 128
                            nc.tensor.matmul(
                                psc[ut][:, 0:n], qt[:, ut * 128:(ut + 1) * 128],
                                qt[:, ut * 128:512],
                                start=(lt == 0), stop=(lt == 31))
                # evict scores (mask diag), bf16
                off = 0
                for ut in range(4):
                    n = 512 - ut * 128
                    nc.vector.tensor_tensor(scsb[b][:, off:off + 128],
                                            psc[ut][:, 0:128], maskU[:], op=OP.mult)
                    if n > 128:
                        nc.scalar.copy(scsb[b][:, off + 128:off + n],
                                       psc[ut][:, 128:n])
                    off += n
                # attn half: [t-tile, d] accumulated over u tiles
                for tt in range(4):
                    pa = ps.tile([128, 512], f32, tag="hot1", bufs=2, name="pa")
                    for ut in range(tt + 1):
                        off = sum(512 - 128 * j for j in range(ut))
                        lhs = scsb[b][:, off + (tt - ut) * 128:off + (tt - ut) * 128 + 128]
                        rhs = xbf[:, (b * 4 + ut) * 256:(b * 4 + ut) * 256 + 256]
                        nc.tensor.matmul(pa[:, 0:256], lhs, rhs, start=(ut == 0),
                                         stop=(ut == tt))
                    j = b * 4 + tt
                    nc.scalar.copy(attn[:, j * 256:(j + 1) * 256], pa[:, 0:256])

            # -- pair AllReduce of attn halves (bf16) --
            if ablate == "att":
                nc.vector.memset(attn[:], 0.001)
            if ablate not in ("att", "noar"):
                a_src = dram.tile([1024, 256], bf16, tag="asrc")
                a_dst = dram.tile([1024, 256], bf16, tag="adst")
                for j in range(NBT):
                    nc.sync.dma_start(a_src[j * 128:(j + 1) * 128, :],
                                      attn[:, j * 256:(j + 1) * 256])
                nc.gpsimd.collective_compute(
                    "AllReduce", OP.add,
                    replica_groups=[[0, 1], [2, 3], [4, 5], [6, 7]],
                    ins=[a_src.opt()], outs=[a_dst.opt()])

            # -- P4: Hebbian chunks (overlaps the AR) --
            # x_shift tiles for chunk 0 (rows u-1 -> partition u-1... see notes)
            for c2 in range(B):
                nc.vector.memset(xsh[c2][96:128, :], 0.0)
                nc.sync.dma_start(xsh[c2][0:127, :],
                                  xbf[1:128, (c2 * 4) * 256:(c2 * 4) * 256 + 256])
            for k in ([] if ablate == "heb" else range(NCH)):
                # transposes: xbt[c2] partition p = xs time (k*128-1+p) [shifted]
                for c2 in range(B):
                    if k == 0:
                        nc.vector.memset(xbt[c2][96:128, :], 0.0)
                    for lt4 in range(NLT // 4):
                        pt = ps.tile([128, 512], bf16, tag="hot0", bufs=2, name="pt4")
                        for q4 in range(4):
                            lt = lt4 * 4 + q4
                            base = lt * 1024 + c2 * 512 + k * 128 - 1
                            if k == 0:
                                src = xs[:, lt * 1024 + c2 * 512:lt * 1024 + c2 * 512 + 127]
                                nc.tensor.transpose(pt[0:127, q4 * 128:q4 * 128 + 128],
                                                    src, ident[:])
                            else:
                                nc.tensor.transpose(pt[:, q4 * 128:q4 * 128 + 128],
                                                    xs[:, base:base + 128], ident[:])
                        rows = slice(0, 127) if k == 0 else slice(0, 128)
                        nc.vector.tensor_copy(xbt[c2][rows, lt4 * 512:lt4 * 512 + 512],
                                              pt[rows, :])
                mS = maskS0 if k == 0 else maskS
                xrhs = [xsh[c2] if k == 0 else None for c2 in range(B)]
                for b in range(B):
                    # H term + S intra into one psum bank
                    ph = pacc.tile([128, 512], f32, tag=f"acc{b}", bufs=1, name="ph")
                    for lt in range(NLT):
                        nc.tensor.matmul(
                            ph[:, 0:256],
                            xs[:, lt * 1024 + b * 512 + k * 128:lt * 1024 + b * 512 + k * 128 + 128],
                            F[:, lt * 256:(lt + 1) * 256],
                            start=(lt == 0), stop=False)
                    # S^T blocks [u,t] for both c2 in one bank
                    pst = ps.tile([128, 512], f32, tag="hot1", bufs=2, name="pst")
                    for c2 in range(B):
                        for lt in range(NLT):
                            if k == 0:
                                lhs = xs[:, lt * 1024 + c2 * 512:lt * 1024 + c2 * 512 + 127]
                                m = 127
                            else:
                                base = lt * 1024 + c2 * 512 + k * 128 - 1
                                lhs = xs[:, base:base + 128]
                                m = 128
                            nc.tensor.matmul(
                                pst[0:m, c2 * 128:c2 * 128 + 128], lhs,
                                xs[:, lt * 1024 + b * 512 + k * 128:lt * 1024 + b * 512 + k * 128 + 128],
                                start=(lt == 0), stop=(lt == NLT - 1))
                    for c2 in range(B):
                        nc.vector.tensor_tensor(ssb[b][:, c2 * 128:c2 * 128 + 128],
                                                pst[:, c2 * 128:c2 * 128 + 128],
                                                mS[:], op=OP.mult)
                    # S apply into same psum as H
                    for c2 in range(B):
                        rhs = (xsh[c2][:] if k == 0
                               else xbf[:, (c2 * 4 + k) * 256:(c2 * 4 + k) * 256 + 256])
                        nc.tensor.matmul(ph[:, 0:256], ssb[b][:, c2 * 128:c2 * 128 + 128],
                                         rhs, start=False, stop=(c2 == B - 1))
                    j = b * 4 + k
                    nc.scalar.copy(ymlp[:, j * 256:(j + 1) * 256], ph[:, 0:256])
                # dF and F update
                for lt in range(NLT):
                    pdf = ps.tile([128, 512], f32, tag="hot0", bufs=2, name="pdf")
                    for c2 in range(B):
                        rhs = (xsh[c2][:] if k == 0
                               else xbf[:, (c2 * 4 + k) * 256:(c2 * 4 + k) * 256 + 256])
                        nc.tensor.matmul(pdf[:, 0:256],
                                         xbt[c2][:, lt * 128:(lt + 1) * 128], rhs,
                                         start=(c2 == 0), stop=(c2 == B - 1))
                    nc.vector.scalar_tensor_tensor(
                        F[:, lt * 256:(lt + 1) * 256], pdf[:, 0:256], LR,
                        F[:, lt * 256:(lt + 1) * 256], op0=OP.mult, op1=OP.add)

            # -- P5: attn AR result -> LN -> y_kv -> y_kv^T --
            if ablate not in ("att", "noar"):
                for j in range(NBT):
                    nc.sync.dma_start(attn[:, j * 256:(j + 1) * 256],
                                      a_dst[j * 128:(j + 1) * 128, :])
            for j in range(NBT):
                emit_ln(attn[:, j * 256:(j + 1) * 256],
                        [ykv[:, j * 256:(j + 1) * 256]])
            for j in range(NBT):
                for dh in range(2):
                    transpose_128(ykvT[:, dh * 1024 + j * 128:dh * 1024 + j * 128 + 128],
                                  ykv[:, j * 256 + dh * 128:j * 256 + dh * 128 + 128],
                                  bf16)

            # -- P6: ys, xy, decoder accumulation --
            pdec = [pacc.tile([128, 512], f32, tag=f"acc{i}", bufs=1, name=f"pdec{i}") for i in range(4)]
            for lt in range(NLT):
                wv = wp.tile([128, 256], bf16, tag="wencv")
                for dh in range(2):
                    nc.sync.dma_start(wv[:, dh * 128:(dh + 1) * 128],
                                      wencv[(lt * 2 + dh) * 128:(lt * 2 + dh + 1) * 128, :])
                wd = wp.tile([128, 256], bf16, tag="wdec")
                nc.sync.dma_start(wd[:], wdec[lt * 128:(lt + 1) * 128, :])
                ys = ysp.tile([128, 1024], bf16, tag="ys")
                for bh in range(2):
                    pp = ps.tile([128, 512], f32, tag="hot0", bufs=2, name="pp2")
                    for dh in range(2):
                        nc.tensor.matmul(
                            pp[:], wv[:, dh * 128:(dh + 1) * 128],
                            ykvT[:, dh * 1024 + bh * 512:dh * 1024 + bh * 512 + 512],
                            start=(dh == 0), stop=(dh == 1))
                    dst = ys[:, bh * 512:(bh + 1) * 512]
                    if lt % 2 == 0:
                        nc.scalar.activation(dst, pp[:], AF.Relu)
                    else:
                        nc.vector.tensor_scalar_max(dst, pp[:], 0.0)
                nc.gpsimd.tensor_tensor(ys[:], ys[:], xs[:, lt * 1024:(lt + 1) * 1024],
                                        op=OP.mult)
                for j in range(NBT):
                    nc.tensor.matmul(pdec[j // 2][:, (j % 2) * 256:(j % 2) * 256 + 256],
                                     ys[:, j * 128:(j + 1) * 128], wd[:],
                                     start=(lt == 0), stop=(lt == NLT - 1))

            # -- P7: finalize y_mlp partial, 8-core AR, residual+LN --
            for j in range(NBT):
                if ablate == "heb":
                    nc.vector.tensor_copy(
                        ymlp[:, j * 256:(j + 1) * 256],
                        pdec[j // 2][:, (j % 2) * 256:(j % 2) * 256 + 256])
                else:
                    nc.vector.tensor_tensor(
                        ymlp[:, j * 256:(j + 1) * 256],
                        pdec[j // 2][:, (j % 2) * 256:(j % 2) * 256 + 256],
                        ymlp[:, j * 256:(j + 1) * 256], op=OP.add)
            if ablate != "noar":
                y_src = dram.tile([1024, 256], bf16, tag="ysrc")
                y_dst = dram.tile([1024, 256], bf16, tag="ydst")
                for j in range(NBT):
                    nc.sync.dma_start(y_src[j * 128:(j + 1) * 128, :],
                                      ymlp[:, j * 256:(j + 1) * 256])
                nc.gpsimd.collective_compute(
                    "AllReduce", OP.add, replica_groups=[list(range(NCORE))],
                    ins=[y_src.opt()], outs=[y_dst.opt()])
                for j in range(NBT):
                    nc.sync.dma_start(ymlp[:, j * 256:(j + 1) * 256],
                                      y_dst[j * 128:(j + 1) * 128, :])
            if taps and lay == 0:
                nc.sync.dma_start(tap_t["t_ymlp"][:], ymlp[:])
                nc.sync.dma_start(tap_t["t_attn"][:], attn[:])
            for j in range(NBT):
                lny = sm.tile([128, 256], f32, tag="lny")
                emit_ln(ymlp[:, j * 256:(j + 1) * 256], [lny[:]])
                z = sm.tile([128, 256], f32, tag="z")
                nc.vector.tensor_tensor(z[:], lny[:], xf[:, j * 256:(j + 1) * 256],
                                        op=OP.add)
                emit_ln(z[:], [xf[:, j * 256:(j + 1) * 256],
                               xbf[:, j * 256:(j + 1) * 256]])
            make_xT()
            if taps and lay == 0:
                nc.sync.dma_start(tap_t["t_x1"][:], xf[:])

        if taps:
            nc.sync.dma_start(tap_t["t_f"][:], F[:])

        # ---------------- lm head ----------------
        lsb = sm.tile([128, 260], bf16, tag="lmh", bufs=1)
        for dh in range(2):
            nc.sync.dma_start(lsb[:, dh * 130:(dh + 1) * 130],
                              lmh[dh * 128:(dh + 1) * 128, :])
        for j in range(NBT):
            pl = ps.tile([128, 512], f32, tag="hot0", bufs=2, name="pl")
            for dh in range(2):
                nc.tensor.matmul(pl[:, 0:130],
                                 xTbf[:, dh * 1024 + j * 128:dh * 1024 + j * 128 + 128],
                                 lsb[:, dh * 130:(dh + 1) * 130],
                                 start=(dh == 0), stop=(dh == 1))
            lg = sm.tile([128, 130], f32, tag="lg")
            nc.scalar.copy(lg[:], pl[:, 0:130])
            nc.sync.dma_start(out[j * 128:(j + 1) * 128, :], lg[:])

    return nc


def _get_nc(n_layer=N_LAYER, taps=False, ablate=None):
    key = (n_layer, taps, ablate)
    if key not in _CACHE:
        nc = bacc.Bacc("TRN2", target_bir_lowering=False, debug=False,
                       num_devices=NCORE)
        _emit(nc, n_layer, taps, ablate)
        nc.compile()
        _CACHE[key] = nc
    return _CACHE[key]


# ------------------------------------------------------------- spmd runner --
# One jitted shard_map executable, built once and reused for every run.
# (Loading a second copy of an 8-core collective executable can desync the
# terminal-side mesh under axon, so kernel() and any timing loop MUST share
# this runner.)
class _Runner:
    def __init__(self, nc):
        import jax
        import concourse.mybir as _mybir
        from jax.sharding import Mesh, PartitionSpec, NamedSharding
        from jax.experimental.shard_map import shard_map
        from concourse import bass2jax
        from concourse.bass2jax import _bass_exec_p, install_neuronx_cc_hook

        install_neuronx_cc_hook()
        self.jax = jax
        self.nc = nc
        part_name = (nc.partition_id_tensor.name
                     if nc.partition_id_tensor else None)
        in_names, out_names, out_avals, zero_outs = [], [], [], []
        for alloc in nc.m.functions[0].allocations:
            if not isinstance(alloc, _mybir.MemoryLocationSet):
                continue
            name = alloc.memorylocations[0].name
            if alloc.kind == "ExternalInput":
                if name != part_name:
                    in_names.append(name)
            elif alloc.kind == "ExternalOutput":
                out_names.append(name)
                shape = tuple(alloc.tensor_shape)
                dtype = _mybir.dt.np(alloc.dtype)
                out_avals.append(jax.core.ShapedArray(shape, dtype))
                zero_outs.append(np.zeros(shape, dtype))
        self.in_names, self.out_names = in_names, out_names
        self.out_avals, self.zero_outs = out_avals, zero_outs
        n_params = len(in_names)
        all_names = list(in_names) + list(out_names)
        if part_name is not None:
            all_names.append(part_name)

        def _body(*args):
            operands = list(args)
            if part_name is not None:
                operands.append(bass2jax.partition_id_tensor())
            outs = _bass_exec_p.bind(
                *operands, out_avals=tuple(out_avals),
                in_names=tuple(all_names), out_names=tuple(out_names),
                lowering_input_output_aliases=(),
                sim_require_finite=True, sim_require_nnan=True, nc=nc)
            return tuple(outs)

        devices = jax.devices()[:NCORE]
        assert len(devices) == NCORE
        self.mesh = Mesh(np.asarray(devices), ("core",))
        self.sharding = NamedSharding(self.mesh, PartitionSpec("core"))
        in_specs = (PartitionSpec("core"),) * (n_params + len(out_names))
        out_specs = (PartitionSpec("core"),) * len(out_names)
        donate = tuple(range(n_params, n_params + len(out_names)))
        self.fn = jax.jit(
            shard_map(_body, mesh=self.mesh, in_specs=in_specs,
                      out_specs=out_specs, check_rep=False),
            donate_argnums=donate, keep_unused=True)

    def device_inputs(self, in_maps):
        concat = [np.concatenate([np.asarray(m[n]) for m in in_maps], 0)
                  for n in self.in_names]
        return [self.jax.device_put(a, self.sharding) for a in concat]

    def zeros_set(self):
        return [self.jax.device_put(
                    np.zeros((NCORE * z.shape[0], *z.shape[1:]), z.dtype),
                    self.sharding)
                for z in self.zero_outs]

    def run(self, dev_in, zeros):
        return self.fn(*dev_in, *zeros)


def _get_runner():
    if "runner" not in _CACHE:
        _CACHE["runner"] = _Runner(_get_nc())
    return _CACHE["runner"]


# -------------------------------------------------------------- host side --
def _perm_local():
    p = np.empty(SH, np.int64)
    p[:2048] = 2 * np.arange(2048)
    p[2048:] = 2 * np.arange(2048) + 1
    return p


def host_prep(idx, embed_w, encoder, encoder_v, decoder, lm_head):
    idx = np.asarray(idx).astype(np.int64)
    embed_w = np.asarray(embed_w, np.float32)
    encoder = np.asarray(encoder, np.float32)
    encoder_v = np.asarray(encoder_v, np.float32)
    decoder = np.asarray(decoder, np.float32)
    lm_head = np.asarray(lm_head, np.float32)
    perm = _perm_local()

    onehotT = np.zeros((256, 1024), np.float32)
    flat = idx.reshape(-1)
    onehotT[flat, np.arange(1024)] = 1.0
    embedp = np.zeros((256, 256), np.float32)
    embedp[:VOCAB] = embed_w

    masks = np.zeros((128, 384), np.float32)
    i = np.arange(128)
    masks[:, 0:128] = (i[None, :] > i[:, None]).astype(np.float32)      # maskU[u,t]
    masks[:, 128:256] = LR * (i[:, None] < i[None, :]).astype(np.float32)  # maskS
    # maskS0: partition p = u-1 (u=p+1); cond u<t -> p+1<t ; row 127 -> 0
    m0 = LR * ((i[:, None] + 1) < i[None, :]).astype(np.float32)
    m0[127] = 0.0
    masks[:, 256:384] = m0

    lmh = lm_head.astype(BF)

    in_maps = []
    for c in range(NCORE):
        h, half = c // 2, c % 2
        g = half * SH + perm
        we = encoder[h][:, g]            # [256, 4096]
        wv = encoder_v[h][:, g]
        dec = decoder[h * L + g, :]      # [4096, 256]
        # tile wenc/wencv: [(lt,dh) * 128, 128]
        wet = np.ascontiguousarray(
            we.reshape(2, 128, NLT, 128).transpose(2, 0, 1, 3).reshape(NLT * 256, 128)
        ).astype(BF)
        wvt = np.ascontiguousarray(
            wv.reshape(2, 128, NLT, 128).transpose(2, 0, 1, 3).reshape(NLT * 256, 128)
        ).astype(BF)
        pg = half * 2048 + np.arange(2048)
        freq = (1.0 / (THETA ** ((2.0 * pg) / L))) / TWO_PI
        ph = np.mod(np.arange(T)[None, :].astype(np.float64)
                    * freq[:, None].astype(np.float64), 1.0) * TWO_PI
        trig = np.concatenate([np.cos(ph), np.sin(ph)], 1).astype(BF)  # [2048,1024]
        in_maps.append({
            "onehotT": onehotT, "embedp": embedp,
            "wenc": wet, "wencv": wvt,
            "wdec": np.ascontiguousarray(dec).astype(BF),
            "trig": np.ascontiguousarray(trig),
            "masks": masks, "lmh": lmh,
        })
    return in_maps


def kernel(idx, embed_w, encoder, encoder_v, decoder, lm_head,
           n_layer=N_LAYER, taps=False, ablate=None, _return_raw=False):
    in_maps = host_prep(idx, embed_w, encoder, encoder_v, decoder, lm_head)
    if n_layer != N_LAYER or taps or ablate or _return_raw:
        # debug path (fresh executable each call — do not mix with timing)
        nc = _get_nc(n_layer, taps, ablate)
        r = run_bass_kernel_spmd(nc, in_maps, core_ids=list(range(NCORE)))
        if _return_raw:
            return r
        return np.ascontiguousarray(
            r.results[0]["out"].reshape(B, T, VOCAB).astype(np.float32))
    rn = _get_runner()
    outs = rn.run(rn.device_inputs(in_maps), rn.zeros_set())
    i = rn.out_names.index("out")
    full = np.asarray(outs[i]).reshape(NCORE, *rn.out_avals[i].shape)[0]
    return np.ascontiguousarray(
        full.reshape(B, T, VOCAB).astype(np.float32))

